# revision 15
# baseline (speedup 1.0000x reference)
"""GroupedQueryAttention Trainium2 kernel (8 NeuronCores), v2.

Sharding: core c -> (batch b = c//4, kv-group g = c%4).
Each core computes its group's 4 query heads over its batch (projections
+ causal attention, all in SBUF), then a PARTIAL output projection over
the FULL d_out using only its own heads' context. A per-s-strip bf16
ReduceScatter over the 4 cores of the batch sums the partials and
scatters s-ranges, so each core emits 4 strips of 128 output rows.
vs v1 (per-head ctx AllGather + column-sharded out-proj): one collective
per s-strip instead of 4 serialized AllGathers, ~4x less collective
traffic, and the RS pipelines behind the next strip's compute.

Layout trick (kept from v1): scores are computed transposed (S^T[k, q])
so A^T = exp(S^T) is directly the lhsT of the ctx matmul; the softmax
denominator is a 129th "ones" column appended to V; normalization is a
per-partition scale of ctx[q, :]. V is projected transposed (V^T, wide
N=512 matmuls) and flipped back with PE transposes.

Bias is seeded via the bo input, which is ZERO for cores with g != 0 —
the program adds it unconditionally, the data makes it correct once.

All matmul operands are bf16 (fp32 PSUM accumulation); the RS runs in
bf16. Measured end-to-end max rel err vs the fp32 reference ~4e-3.
"""

from contextlib import ExitStack

import numpy as np
import ml_dtypes

import concourse.bass as bass
import concourse.bacc as bacc
import concourse.tile as tile
from concourse import mybir
from concourse.bass_utils import run_bass_kernel_spmd
from concourse.masks import make_identity
from concourse.tile_rust import add_dep_helper

BF16 = mybir.dt.bfloat16
F32 = mybir.dt.float32

B = 2
S = 2048
D = 2048
G = 4  # kv groups
HPG = 4  # heads per group
HD = 128  # head dim
QC = 512  # q-chunk (columns per S^T block / s-strip rows)
NQC = S // QC  # 4
NKT = S // 128  # 16 k-tiles
NDC = D // 128  # 16 d_in chunks
SCALE = 1.0 / float(np.sqrt(HD))
N_CORES = 8
REPLICA_GROUPS = [[0, 1, 2, 3], [4, 5, 6, 7]]


def _build_program():
    nc = bacc.Bacc("TRN2", target_bir_lowering=False, debug=True)

    xq = nc.declare_dram_parameter("xq", [NQC, NDC, 128, QC], BF16, isOutput=False)
    wq = nc.declare_dram_parameter("wq", [HPG, 128, NDC, HD], BF16, isOutput=False)
    wk = nc.declare_dram_parameter("wk", [128, NDC, HD], BF16, isOutput=False)
    wv = nc.declare_dram_parameter("wv", [128, NDC, HD], BF16, isOutput=False)
    wo = nc.declare_dram_parameter("wo", [128, HPG, D], BF16, isOutput=False)
    bo = nc.declare_dram_parameter("bo", [1, D], BF16, isOutput=False)
    msk = nc.declare_dram_parameter("msk", [128, 896], BF16, isOutput=False)
    # bf16 output: the RS result is already bf16-rounded; the host upcasts.
    out_ext = nc.declare_dram_parameter("out", [NQC, 128, D], BF16, isOutput=True)

    # ReduceScatter outputs (one per s-strip)
    rs_out = [nc.dram_tensor(f"rso{qc}", [128, D], BF16) for qc in range(NQC)]

    with tile.TileContext(nc) as tc, ExitStack() as es:
        singles = es.enter_context(tc.tile_pool(name="singles", bufs=1))
        wpool = es.enter_context(tc.tile_pool(name="w", bufs=1))
        xpool = es.enter_context(tc.tile_pool(name="x", bufs=2))
        qkpool = es.enter_context(tc.tile_pool(name="qk", bufs=1))
        apool = es.enter_context(tc.tile_pool(name="a", bufs=32))
        spool = es.enter_context(tc.tile_pool(name="sm", bufs=4))
        cpool = es.enter_context(tc.tile_pool(name="cs", bufs=6))
        opool = es.enter_context(tc.tile_pool(name="o", bufs=2))
        ps_big = es.enter_context(tc.tile_pool(name="psb", bufs=4, space="PSUM"))
        ps_small = es.enter_context(tc.tile_pool(name="pss", bufs=4, space="PSUM"))
        dram = es.enter_context(tc.tile_pool(name="dram", bufs=1, space="DRAM"))

        # --- first-strip x plus K/V/Q weights, in compute order, so the
        # first matmuls start as early as possible. wo/msk/bias follow.
        xstrip0 = xpool.tile([128, NDC, QC], BF16, tag="xs")
        nc.sync.dma_start(out=xstrip0, in_=xq[0].rearrange("a p q -> p a q"))
        # weights arrive pre-packed partition-major from the host so every
        # DMA reads >=4KB contiguous runs (256B runs halve DMA throughput)
        wkall = wpool.tile([128, NDC, HD], BF16, tag="wkall")
        nc.sync.dma_start(out=wkall, in_=wk[:, :, :])
        wvall = wpool.tile([128, NDC, HD], BF16, tag="wvall")
        nc.sync.dma_start(out=wvall, in_=wv[:, :, :])
        woall = wpool.tile([128, HPG, D], BF16, tag="woall")
        # per-head Wq tiles: head h's projection starts after 512KB, not 2MB
        wqh = []
        for h in range(HPG):
            wqt = wpool.tile([128, NDC, HD], BF16, tag=f"wq{h}", name=f"wq{h}")
            nc.sync.dma_start(out=wqt, in_=wq[h])
            wqh.append(wqt)

        ident = singles.tile([128, 128], BF16, tag="ident")
        make_identity(nc, ident)
        ones1 = singles.tile([1, 128], BF16, tag="ones1")
        nc.vector.memset(ones1, 1.0)
        mask_sb = singles.tile([128, 896], BF16, tag="mask")
        nc.sync.dma_start(out=mask_sb, in_=msk[:, :])
        bo_sb = singles.tile([1, D], BF16, tag="bo")
        nc.sync.dma_start(out=bo_sb, in_=bo[:, :])
        nc.sync.dma_start(out=woall, in_=wo[:, :, :])
        bias_sb = singles.tile([128, D], BF16, tag="bias")

        wk_sb = [wkall[:, dc, :] for dc in range(NDC)]
        wv_sb = [wvall[:, dc, :] for dc in range(NDC)]

        # --- persistent activations ---
        qT = [qkpool.tile([128, S], BF16, tag=f"qT{h}", name=f"qT{h}") for h in range(HPG)]
        kT = qkpool.tile([128, S], BF16, tag="kT")
        vext = [
            qkpool.tile([128, HD + 1], BF16, tag=f"v{i}", name=f"v{i}")
            for i in range(NKT)
        ]

        colls = []
        for qc in range(NQC):
            # ======== projections for this s-strip ========
            if qc == 0:
                xstrip = xstrip0
            else:
                xstrip = xpool.tile([128, NDC, QC], BF16, tag="xs")
                nc.sync.dma_start(
                    out=xstrip, in_=xq[qc].rearrange("a p q -> p a q")
                )
            xs = [xstrip[:, dc, :] for dc in range(NDC)]
            # K^T first (smallest weight DMA): [dh, k 512]
            ps = ps_big.tile([128, QC], F32, tag="big")
            for dc in range(NDC):
                nc.tensor.matmul(
                    ps,
                    lhsT=wk_sb[dc],
                    rhs=xs[dc],
                    start=(dc == 0),
                    stop=(dc == NDC - 1),
                )
            nc.vector.tensor_copy(kT[:, qc * QC : (qc + 1) * QC], ps)
            # Q^T per head: [dh=128, q 512]
            for h in range(HPG):
                ps = ps_big.tile([128, QC], F32, tag="big")
                for dc in range(NDC):
                    nc.tensor.matmul(
                        ps,
                        lhsT=wqh[h][:, dc, :],
                        rhs=xs[dc],
                        start=(dc == 0),
                        stop=(dc == NDC - 1),
                    )
                nc.vector.tensor_copy(qT[h][:, qc * QC : (qc + 1) * QC], ps)
            # V^T: [dv, s 512] with wide matmuls, then PE-transpose into
            # the [k 128, dv] tiles the ctx matmul wants.
            ps = ps_big.tile([128, QC], F32, tag="big")
            for dc in range(NDC):
                nc.tensor.matmul(
                    ps,
                    lhsT=wv_sb[dc],
                    rhs=xs[dc],
                    start=(dc == 0),
                    stop=(dc == NDC - 1),
                )
            vt_sb = cpool.tile([128, QC], BF16, tag="vt")
            nc.vector.tensor_copy(vt_sb, ps)
            for st in range(4):
                kt = qc * 4 + st
                tp = ps_small.tile([128, 128], BF16, tag="tp", bufs=2)
                nc.tensor.transpose(tp, vt_sb[:, st * 128 : (st + 1) * 128], ident)
                nc.vector.tensor_copy(vext[kt][:, 0:HD], tp)
                nc.vector.memset(vext[kt][:, HD : HD + 1], 1.0)

            # ======== attention for this strip's 4 heads ========
            ctq = [
                cpool.tile([128, QC], BF16, tag=f"ctq{h}", name=f"ctq{h}")
                for h in range(HPG)
            ]
            nkt = 4 * qc + 4  # causal: k-tiles 0 .. 4qc+3
            for h in range(HPG):
                a_blocks = []
                for kt in range(nkt):
                    ps = ps_big.tile([128, QC], F32, tag="big")
                    nc.tensor.matmul(
                        ps,
                        lhsT=kT[:, kt * 128 : (kt + 1) * 128],
                        rhs=qT[h][:, qc * QC : (qc + 1) * QC],
                        start=True,
                        stop=True,
                    )
                    a = apool.tile([128, QC], BF16, tag="a")
                    nc.scalar.activation(
                        out=a,
                        in_=ps,
                        func=mybir.ActivationFunctionType.Exp,
                        scale=SCALE,
                    )
                    if kt >= 4 * qc:  # diagonal block: causal mask (post-exp)
                        off = 128 * kt - 512 * qc
                        nc.vector.tensor_mul(
                            a, a, mask_sb[:, 384 - off : 384 - off + QC]
                        )
                    a_blocks.append(a)
                for st in range(4):
                    qt = qc * 4 + st
                    cps = ps_small.tile([128, HD + 1], F32, tag="small", bufs=2)
                    for kt in range(qt + 1):
                        nc.tensor.matmul(
                            cps,
                            lhsT=a_blocks[kt][:, st * 128 : (st + 1) * 128],
                            rhs=vext[kt],
                            start=(kt == 0),
                            stop=(kt == qt),
                        )
                    zr = cpool.tile([128, 1], F32, tag="zr")
                    nc.vector.reciprocal(zr, cps[:, HD : HD + 1])
                    cs = cpool.tile([128, HD], BF16, tag="cs")
                    nc.vector.tensor_scalar_mul(cs, cps[:, 0:HD], zr)
                    tp = ps_small.tile([128, 128], BF16, tag="tp", bufs=2)
                    nc.tensor.transpose(tp, cs, ident)
                    nc.vector.tensor_copy(ctq[h][:, st * 128 : (st + 1) * 128], tp)

            if qc == 0:
                # bias broadcast to all 128 partitions (bo is zeros on
                # g!=0 cores). Emitted here, not at the top: the PE queue
                # is strict in-order, so emitting these first would stall
                # every matmul behind the bo DMA.
                for cc in range(4):
                    bps = ps_big.tile([128, QC], F32, tag="big")
                    nc.tensor.matmul(
                        bps,
                        lhsT=ones1,
                        rhs=bo_sb[:, cc * QC : (cc + 1) * QC],
                        start=True,
                        stop=True,
                    )
                    nc.vector.tensor_copy(bias_sb[:, cc * QC : (cc + 1) * QC], bps)

            # ======== partial out-proj for s-strip qc (full d_out) ========
            pout = dram.tile([QC, D], BF16, tag="pout", bufs=2)
            writes = []
            for st in range(4):
                for cc in range(4):
                    ps = ps_big.tile([128, QC], F32, tag="big")
                    for h in range(HPG):
                        nc.tensor.matmul(
                            ps,
                            lhsT=ctq[h][:, st * 128 : (st + 1) * 128],
                            rhs=woall[:, h, cc * QC : (cc + 1) * QC],
                            start=(h == 0),
                            stop=(h == HPG - 1),
                        )
                    posb = spool.tile([128, QC], BF16, tag="posb")
                    nc.vector.tensor_add(
                        posb, ps, bias_sb[:, cc * QC : (cc + 1) * QC]
                    )
                    w = nc.sync.dma_start(
                        out=pout[
                            st * 128 : (st + 1) * 128,
                            cc * QC : (cc + 1) * QC,
                        ],
                        in_=posb,
                    )
                    writes.append(w)

            coll = nc.gpsimd.collective_compute(
                "ReduceScatter",
                mybir.AluOpType.add,
                replica_groups=REPLICA_GROUPS,
                ins=[pout[:, :].opt()],
                outs=[rs_out[qc][:, :].opt()],
            )
            for w in writes:
                add_dep_helper(coll.ins, w.ins, reason="pout-write->rs")
            colls.append(coll)

            # ======== emit this strip's RS result (DRAM->DRAM copy) ========
            d = nc.sync.dma_start(out=out_ext[qc], in_=rs_out[qc][:, :])
            add_dep_helper(d.ins, coll.ins, reason="rs->emit")

    nc.compile()
    return nc


def _make_mask() -> np.ndarray:
    # base[k, j] = 1.0 if (j - 384) >= k else 0; diag block with offset
    # `off` uses columns [384-off : 896-off]: mask[k, q'] = (q' >= k + off).
    j = np.arange(896)[None, :]
    k = np.arange(128)[:, None]
    return ((j - 384) >= k).astype(ml_dtypes.bfloat16)


def _make_in_maps(inputs) -> list[dict]:
    x = np.asarray(inputs["x"], dtype=np.float32)
    Wq = np.asarray(inputs["Wq"], dtype=np.float32)
    Wk = np.asarray(inputs["Wk"], dtype=np.float32)
    Wv = np.asarray(inputs["Wv"], dtype=np.float32)
    Wo = np.asarray(inputs["Wo"], dtype=np.float32)
    bo = np.asarray(inputs["bo"], dtype=np.float32)

    bf = ml_dtypes.bfloat16
    mask = _make_mask()
    bo_bf = bo.astype(bf).reshape(1, D)
    bo_zero = np.zeros((1, D), dtype=bf)

    # x^T tiled: [qc, dc, 128, 512] per batch
    xqs = []
    for b in range(B):
        xT = np.ascontiguousarray(x[b].T.astype(bf))  # [d, s]
        xqs.append(
            np.ascontiguousarray(xT.reshape(NDC, 128, NQC, QC).transpose(2, 0, 1, 3))
        )

    in_maps = []
    for c in range(N_CORES):
        b, g = c // 4, c % 4
        # partition-major packs so every weight DMA reads >=4KB runs:
        #   wq[h, p, dc, i] = Wq[dc*128+p, (g*4+h)*128+i]
        #   wk[p, dc, i]    = Wk[dc*128+p, g*128+i]        (wv alike)
        #   wo[p, h, :]     = Wo[(g*4+h)*128+p, :]
        wq_c = np.ascontiguousarray(
            Wq[:, g * 512 : (g + 1) * 512]
            .astype(bf)
            .reshape(NDC, 128, HPG, HD)
            .transpose(2, 1, 0, 3)
        )
        wk_c = np.ascontiguousarray(
            Wk[:, g * HD : (g + 1) * HD].astype(bf).reshape(NDC, 128, HD).transpose(1, 0, 2)
        )
        wv_c = np.ascontiguousarray(
            Wv[:, g * HD : (g + 1) * HD].astype(bf).reshape(NDC, 128, HD).transpose(1, 0, 2)
        )
        wo_c = np.ascontiguousarray(
            Wo[g * 512 : (g + 1) * 512, :].astype(bf).reshape(HPG, 128, D).transpose(1, 0, 2)
        )
        in_maps.append(
            {
                "xq": xqs[b],
                "wq": wq_c,
                "wk": wk_c,
                "wv": wv_c,
                "wo": wo_c,
                "bo": bo_bf if g == 0 else bo_zero,
                "msk": mask,
            }
        )
    return in_maps


def _assemble(results) -> np.ndarray:
    out = np.empty((B, S, D), dtype=np.float32)
    for c in range(N_CORES):
        b, g = c // 4, c % 4
        for qc in range(NQC):
            # device output is bf16 (the RS result is already bf16-rounded);
            # numpy assignment upcasts to the module's fp32 output dtype
            out[b][qc * QC + g * 128 : qc * QC + (g + 1) * 128, :] = results[c][
                "out"
            ][qc].astype(np.float32)
    return out


def kernel(**inputs) -> np.ndarray:
    in_maps = _make_in_maps(inputs)
    nc = _build_program()
    res = run_bass_kernel_spmd(nc, in_maps, list(range(N_CORES)))
    return _assemble(res.results)
